# revision 1
# baseline (speedup 1.0000x reference)
"""Causal single-head attention on 8 Trainium2 NeuronCores.

Problem: x[4096,1024] -> Q,K,V = x@W.T+b (d_k=64), out = softmax(causal(QK^T/8)) @ V.

Strategy (sequence-parallel, uniform SPMD):
  - Query blocks of 128 rows; 32 blocks total. Core c owns global blocks
    {c, c+8, c+16, c+24} (strided) -> every core runs the IDENTICAL program.
  - Each core computes K^T/V~ for its own 512 rows, AllGathers them (split in
    two halves so the second gather overlaps band-0/1 compute), then attends
    its 4 q-blocks against the gathered keys.
  - Band schedule: band s in 0..3 attends q-slots s..3 (N = 512-128s cols)
    against shard-slot s of every rank (8 steps/band). Causality is exact:
    global kb = c'+8s vs qb = c+8j; s<j always valid, s==j masked by a
    per-core host-built mask (ones/triu/zeros by c' vs c), s>j never needed.
  - Softmax denominator comes free: V~ has a ones-column appended, so the
    AV matmul accumulates [out^T; rowsum(E)] in one pass. exp on ScalarE with
    the 1/8 scale folded in; no max-subtraction (scores are O(1) here).
  - float32r (full-rate fp32) matmuls end-to-end; all transposes of inputs
    (x^T, W^T, identity) are host-side numpy; only V^T->V~ (4) and the
    output (4) transpose on-device via PE.
"""

import os
import numpy as np
from contextlib import ExitStack

S, DM, DK = 4096, 1024, 64
NCORES = 8
QB = 128                      # rows per block
SLOTS = 4                     # q-blocks per core
SH = QB * SLOTS               # 512 shard rows per core
# per-half shard (slots 0-1 or 2-3): K^T [64, 256] + V~ [128, 2*65]
KT_H = DK * 2 * QB            # 16384
VT_H = QB * 2 * (DK + 1)      # 16640
SHARD_H = KT_H + VT_H         # 33024

USE_F32R = os.environ.get("KERNEL_F32", "0") != "1"
AMP = int(os.environ.get("KERNEL_AMP", "1"))  # repeat whole pipeline in-NEFF

LAST_EXEC_NS = None


def _build_nc():
    import concourse.bass as bass
    import concourse.bacc as bacc
    import concourse.mybir as mybir
    import concourse.tile as tile

    f32 = mybir.dt.float32
    fr = mybir.dt.float32r if USE_F32R else mybir.dt.float32
    AF = mybir.ActivationFunctionType

    nc = bacc.Bacc(None, num_devices=NCORES)

    xT_d = nc.dram_tensor("xT", [DM, SH], fr, kind="ExternalInput")
    wqkT_d = nc.dram_tensor("wqkT", [DM, 2 * DK], fr, kind="ExternalInput")
    wvT_d = nc.dram_tensor("wvT", [DM, DK], fr, kind="ExternalInput")
    bqk_d = nc.dram_tensor("bqk", [2 * DK, 1], f32, kind="ExternalInput")
    bv_d = nc.dram_tensor("bv", [DK, 1], f32, kind="ExternalInput")
    mask_d = nc.dram_tensor("mask", [NCORES * QB, SH], fr, kind="ExternalInput")
    tri_d = nc.dram_tensor("tri", [QB, QB], fr, kind="ExternalInput")
    ident_d = nc.dram_tensor("ident", [128, 128], fr, kind="ExternalInput")
    out_d = nc.dram_tensor("out", [SH, DK], f32, kind="ExternalOutput")

    with tile.TileContext(nc) as tc, ExitStack() as ctx:
        singles = ctx.enter_context(tc.tile_pool(name="singles", bufs=1))
        psum = ctx.enter_context(tc.tile_pool(name="psum", bufs=1, space="PSUM"))
        psum2 = ctx.enter_context(tc.tile_pool(name="psum2", bufs=2, space="PSUM"))
        kvpool = ctx.enter_context(tc.tile_pool(name="kvpool", bufs=3))
        epool = ctx.enter_context(tc.tile_pool(name="epool", bufs=3))
        dram = ctx.enter_context(tc.tile_pool(name="dram", bufs=1, space="DRAM"))

        # ---------------- input loads (small constants first) ----------------
        wqk_sb = singles.tile([128, DM // 128, 2 * DK], fr)
        nc.sync.dma_start(out=wqk_sb, in_=wqkT_d[:, :].rearrange("(d p) c -> p d c", p=128))
        wv_sb = singles.tile([128, DM // 128, DK], fr)
        nc.sync.dma_start(out=wv_sb, in_=wvT_d[:, :].rearrange("(d p) c -> p d c", p=128))
        bqk_sb = singles.tile([128, 1], f32)
        nc.sync.dma_start(out=bqk_sb, in_=bqk_d[:, :])
        bv_sb = singles.tile([64, 1], f32)
        nc.sync.dma_start(out=bv_sb, in_=bv_d[:, :])
        mask_sb = singles.tile([128, NCORES, SH], fr)
        tri_sb = singles.tile([128, QB], fr)
        ident_fr = singles.tile([128, 128], fr)
        nc.sync.dma_start(out=ident_fr, in_=ident_d[:, :])

        xT_sb = singles.tile([128, DM // 128, SH], fr)
        qkT_sb = singles.tile([128, SH], fr)
        vt_sb = singles.tile([128, SLOTS, DK + 1], fr)
        # ones column of V~ (f32r memset is invalid ISA; ACT writes 0*x+1)
        nc.scalar.activation(vt_sb[:, :, DK:DK + 1], ident_fr[:, 0:SLOTS].bitcast(f32),
                             AF.Identity, bias=1.0, scale=0.0)
        def load_xt_half(h):
            cs = slice(256 * h, 256 * (h + 1))
            for q in range(2):
                nc.sync.dma_start(
                    out=xT_sb[:, 4 * q:4 * (q + 1), cs],
                    in_=xT_d[512 * q:512 * (q + 1), cs].rearrange(
                        "(d p) s -> p d s", p=128))

        rep_counter = [0]

        def band_kt_ap(ag_out, s):
            t = ag_out[s // 2]
            return bass.AP(tensor=t.tensor, offset=t.offset + QB * (s % 2),
                           ap=[[2 * QB, DK], [SHARD_H, NCORES], [1, QB]])

        def band_vt_ap(ag_out, s):
            t = ag_out[s // 2]
            return bass.AP(tensor=t.tensor,
                           offset=t.offset + KT_H + (DK + 1) * (s % 2),
                           ap=[[2 * (DK + 1), QB], [SHARD_H, NCORES], [1, DK + 1]])

        def one_pass():
            # ------------- per-half: project, build V~, AllGather -------------
            r = rep_counter[0]
            rep_counter[0] += 1
            ag_in = [dram.tile([SHARD_H], fr, name=f"ag_in{r}_{h}",
                               tag=f"agi{r}_{h}") for h in range(2)]
            ag_out = [dram.tile([NCORES * SHARD_H], fr, addr_space="Shared",
                                name=f"ag_out{r}_{h}", tag=f"ago{r}_{h}")
                      for h in range(2)]
            load_xt_half(0)
            for h in range(2):
                cols = slice(256 * h, 256 * (h + 1))
                qk_ps = psum2.tile([128, 256], f32, tag="sc", bufs=3, name="qk_ps")
                v_ps = psum2.tile([64, 256], f32, tag="sc", bufs=3, name="v_ps")
                for d in range(DM // 128):
                    nc.tensor.matmul(qk_ps, lhsT=wqk_sb[:, d, :],
                                     rhs=xT_sb[:, d, cols],
                                     start=(d == 0), stop=(d == DM // 128 - 1))
                    nc.tensor.matmul(v_ps, lhsT=wv_sb[:, d, :],
                                     rhs=xT_sb[:, d, cols],
                                     start=(d == 0), stop=(d == DM // 128 - 1))
                nc.scalar.activation(qkT_sb[:, cols], qk_ps, AF.Identity,
                                     bias=bqk_sb[:, 0:1], scale=1.0)
                nc.sync.dma_start(
                    out=ag_in[h][0:KT_H].rearrange("(p s) -> p s", p=DK),
                    in_=qkT_sb[64:128, cols])
                vT_h = epool.tile([64, 256], fr, tag="vth", name="vT_h")
                nc.scalar.activation(vT_h, v_ps, AF.Identity,
                                     bias=bv_sb[:, 0:1], scale=1.0)
                for sl in range(2):
                    t_ps = psum2.tile([128, 64], fr, tag="tps", bufs=1, name="t_ps")
                    nc.tensor.transpose(t_ps, vT_h[:, 128 * sl:128 * (sl + 1)],
                                        ident_fr[0:64, 0:64])
                    nc.scalar.copy(vt_sb[:, 2 * h + sl, 0:DK], t_ps)
                nc.sync.dma_start(
                    out=ag_in[h][KT_H:SHARD_H].rearrange("(p a) -> p a", p=128),
                    in_=vt_sb[:, 2 * h:2 * (h + 1), :].rearrange("p a b -> p (a b)"))
                nc.gpsimd.collective_compute(
                    "AllGather", mybir.AluOpType.bypass,
                    replica_groups=[list(range(NCORES))],
                    ins=[ag_in[h][:]], outs=[ag_out[h][:]],
                )
                if h == 0:
                    if r == 0:
                        # 2MB of masks: behind both x^T halves; first needed
                        # only by the prepass multiplies
                        nc.sync.dma_start(
                            out=mask_sb,
                            in_=mask_d[:, :].rearrange("(c p) q -> p c q", p=128))
                        nc.sync.dma_start(out=tri_sb, in_=tri_d[:, :])
                    av_ps = psum.tile([DK + 1, SH], f32, name="av_ps")
                    for s in range(2):
                        c0, N = 128 * s, 256 - 128 * s
                        lsc = psum2.tile([128, 2, 512], f32, tag="sc", bufs=3, name="lscA")
                        le = epool.tile([128, 2, 512], fr, tag="e", name="leA")
                        lkt = kvpool.tile([DK, QB], fr, tag="lkt", name="lktA")
                        nc.sync.dma_start(
                            out=lkt,
                            in_=ag_in[0][0:KT_H].rearrange(
                                "(p s) -> p s", p=DK)[:, QB * s:QB * (s + 1)])
                        nc.tensor.matmul(lsc[:, 0, 0:N], lhsT=lkt,
                                         rhs=qkT_sb[0:64, c0:256],
                                         start=True, stop=True)
                        nc.scalar.activation(le[:, 0, 0:N], lsc[:, 0, 0:N],
                                             AF.Exp, scale=0.125)
                        nc.vector.tensor_mul(le[:, 0, 0:QB], le[:, 0, 0:QB],
                                             tri_sb)
                        nc.tensor.matmul(av_ps[:, c0:256], lhsT=vt_sb[:, s, :],
                                         rhs=le[:, 0, 0:N], start=(s == 0),
                                         stop=False, skip_group_check=True)
                    load_xt_half(1)


            # ---- local prepass part B: own blocks vs Q cols 256:512 ----
            # (part A ran inside the h-loop right after half 0; see below)
            for s in range(SLOTS):
                c0 = max(256, 128 * s)
                N = SH - c0
                lsc = psum2.tile([128, 2, 512], f32, tag="sc", bufs=3, name="lscB")
                le = epool.tile([128, 2, 512], fr, tag="e", name="leB")
                lkt = kvpool.tile([DK, QB], fr, tag="lkt", name="lktB")
                nc.sync.dma_start(
                    out=lkt,
                    in_=ag_in[s // 2][0:KT_H].rearrange(
                        "(p s) -> p s", p=DK)[:, QB * (s % 2):QB * (s % 2 + 1)])
                nc.tensor.matmul(lsc[:, 0, 0:N], lhsT=lkt,
                                 rhs=qkT_sb[0:64, c0:SH], start=True, stop=True)
                nc.scalar.activation(le[:, 0, 0:N], lsc[:, 0, 0:N], AF.Exp,
                                     scale=0.125)
                if s >= 2:   # diagonal strip lies in these columns
                    nc.vector.tensor_mul(le[:, 0, 0:QB], le[:, 0, 0:QB], tri_sb)
                nc.tensor.matmul(av_ps[:, c0:SH], lhsT=vt_sb[:, s, :],
                                 rhs=le[:, 0, 0:N], start=False, stop=False,
                                 skip_group_check=True)

            # ---------------- attention bands ----------------
            first_av = False
            for s in range(SLOTS):
                N = SH - 128 * s
                q_ap = qkT_sb[0:64, 128 * s:SH]
                ktb = kvpool.tile([DK, NCORES, QB], fr, tag="ktb", name="ktb")
                vtb = kvpool.tile([QB, NCORES, DK + 1], fr, tag="vtb", name="vtb")
                nc.sync.dma_start(out=ktb, in_=band_kt_ap(ag_out, s))
                nc.sync.dma_start(out=vtb, in_=band_vt_ap(ag_out, s))
                W = 2 if s < 2 else 4       # steps per exp; N<=256 fits 4/tile
                for g in range(NCORES // W):
                    sc_ps = psum2.tile([128, W, 1024 // W], f32, tag="sc",
                                       bufs=3, name="sc_ps")
                    e_sb = epool.tile([128, W, 1024 // W], fr, tag="e", name="e_sb")
                    for hh in range(W):
                        cp = W * g + hh
                        nc.tensor.matmul(sc_ps[:, hh, 0:N], lhsT=ktb[:, cp, :],
                                         rhs=q_ap, start=True, stop=True)
                    nc.scalar.activation(e_sb[:, :, 0:N], sc_ps[:, :, 0:N], AF.Exp,
                                         scale=0.125)
                    for hh in range(W):
                        cp = W * g + hh
                        nc.vector.tensor_mul(e_sb[:, hh, 0:N], e_sb[:, hh, 0:N],
                                             mask_sb[:, cp, 0:N])
                        last_av = (s == SLOTS - 1 and g == NCORES // W - 1
                                   and hh == W - 1)
                        nc.tensor.matmul(av_ps[:, 128 * s:SH], lhsT=vtb[:, cp, :],
                                         rhs=e_sb[:, hh, 0:N],
                                         start=first_av, stop=last_av,
                                         skip_group_check=True)
                        first_av = False

            # ------------- epilogue: transpose, normalize, store -------------
            av_sb = singles.tile([DK + 1, SH], f32, name="av_sb")
            nc.scalar.copy(av_sb, av_ps)
            out_sb = singles.tile([128, SLOTS, DK], f32, name="out_sb")
            for sl in range(SLOTS):
                t2 = psum2.tile([128, DK + 1], f32, tag="tps", bufs=1, name="t2")
                nc.tensor.transpose(t2, av_sb[0:DK + 1, 128 * sl:128 * (sl + 1)],
                                    ident_fr[0:DK + 1, 0:DK + 1].bitcast(f32))
                rec = epool.tile([128, 1], f32, tag="rec", name="rec")
                nc.vector.reciprocal(rec, t2[:, DK:DK + 1])
                nc.vector.tensor_scalar_mul(out_sb[:, sl, :], t2[:, 0:DK], rec)
                nc.sync.dma_start(out=out_d[128 * sl:128 * (sl + 1), :],
                                  in_=out_sb[:, sl, :])

        for _rep in range(AMP):
            one_pass()

    nc.finalize()
    return nc


def _in_maps(x, Wq, bq, Wk, bk, Wv, bv):
    wqkT = np.ascontiguousarray(np.concatenate([Wq.T, Wk.T], axis=1), dtype=np.float32)
    wvT = np.ascontiguousarray(Wv.T, dtype=np.float32)
    bqk = np.concatenate([bq, bk]).reshape(2 * DK, 1).astype(np.float32)
    bvv = bv.reshape(DK, 1).astype(np.float32)
    tri = np.triu(np.ones((QB, QB), dtype=np.float32))  # E^T[k,q] valid iff k<=q
    maps = []
    for c in range(NCORES):
        rows = np.concatenate([np.arange(QB * (c + 8 * sl), QB * (c + 8 * sl) + QB)
                               for sl in range(SLOTS)])
        xT = np.ascontiguousarray(x[rows].T, dtype=np.float32)  # [1024, 512]
        # [c', k, q-col] over the full 512-col band window. strip = first 128
        # cols (q-slot s); own position contributes via the local prepass.
        m = np.zeros((NCORES, QB, SH), dtype=np.float32)
        m[:c] = 1.0                   # earlier ranks: fully valid
        m[c + 1:, :, QB:] = 1.0       # later ranks: valid beyond the strip
        maps.append({
            "xT": xT, "wqkT": wqkT, "wvT": wvT, "bqk": bqk, "bv": bvv,
            "mask": np.ascontiguousarray(m.reshape(NCORES * QB, SH)),
            "tri": tri, "ident": np.eye(128, dtype=np.float32),
        })
    return maps


def kernel(**inputs):
    global LAST_EXEC_NS
    x = np.asarray(inputs["x"], dtype=np.float32)
    args = [np.asarray(inputs[k], dtype=np.float32)
            for k in ("Wq", "bq", "Wk", "bk", "Wv", "bv")]
    in_maps = _in_maps(x, args[0], args[1], args[2], args[3], args[4], args[5])

    nc = _build_nc()
    from concourse.bass_utils import run_bass_kernel_spmd
    res = run_bass_kernel_spmd(nc, in_maps, core_ids=list(range(NCORES)))
    LAST_EXEC_NS = res.exec_time_ns

    out = np.zeros((S, DK), dtype=np.float32)
    for c in range(NCORES):
        r = res.results[c]["out"]
        for sl in range(SLOTS):
            b = c + 8 * sl
            out[QB * b:QB * (b + 1)] = r[QB * sl:QB * (sl + 1)]
    return out



# revision 13
# speedup vs baseline: 2.1345x; 2.1345x over previous
"""Causal single-head attention on 8 Trainium2 NeuronCores — fully local.

Problem: x[4096,1024] -> Q,K,V = x@W.T+b (d_k=64), out = softmax(causal(QK^T/8)) @ V.

Strategy (replicated K/V, zero communication):
  - Every core loads the FULL x^T in bf16 (8 MB) and computes K^T and V for
    all 4096 rows locally; no collective, no cross-core sync of any kind.
    The 8 MB stream overlaps the projection/attention pipeline.
  - Query blocks of 128 rows; core c owns global blocks {c, 8+c, 16+c, 24+c}
    (strided) -> every core runs the IDENTICAL program. Slot j attends key
    blocks 0..8j+7 (uniform); within the diagonal band (blocks 8j..8j+7) a
    per-core host-built mask (ones/tri/zeros by key index vs c) enforces
    exact causality. Off-band blocks are always fully valid -> no masking.
  - x-column chunks of 512 stream in; chunk g yields key blocks 4g..4g+3.
    K^T/V^T come from a stacked [Wk|Wv] projection run as TWO interleaved
    256-column PSUM chains (keeps the PE p-state ramp hot); V~ blocks are PE
    transposes of V^T with a ones-column appended so the AV matmul also
    accumulates the softmax denominator.
  - Attention (scores -> exp -> mask -> AV) for every (q-slot, key-group)
    pair runs as soon as its chunk is projected; AV matmuls are emitted
    round-robin across slots so accumulation chains interleave on the PE.
  - All attention matmuls in bf16 (rate-1 at any moving width); exp on
    ScalarE with the 1/8 scale folded in; accumulation in f32 PSUM.
  - Constants ride in two packed blobs (one bf16, one f32) to amortize
    per-DMA overhead; the x stream is ordered chunk0 -> xq -> chunks 1-7 so
    the projection pipeline starts as early as possible.
"""

import os
import numpy as np
import ml_dtypes
from contextlib import ExitStack

S, DM, DK = 4096, 1024, 64
NCORES = 8
QB = 128                      # rows per block
SLOTS = 4                     # q-blocks per core
SH = QB * SLOTS               # 512 own query rows per core
NB = S // QB                  # 32 key blocks
CHUNK = 512                   # x columns per streamed chunk
NCH = S // CHUNK              # 8 chunks
ND = DM // 128                # 8 contraction chunks

# cb_w bf16 blob (early): wkv [8, 128] | ident [128]
BF_WKV = 0
BF_ID = ND * 128
BFW_COLS = BF_ID + 128
# cb_r bf16 blob (later): wq [8, 64] | mask [8, 128]
BFR_WQ = 0
BFR_MASK = ND * DK
BFR_COLS = BFR_MASK + NCORES * QB
# f32 blob layout: bkv [1] | bq [1] | identf [128]
F_COLS = 2 + 128

AMP = int(os.environ.get("KERNEL_AMP", "1"))  # repeat whole pipeline in-NEFF

LAST_EXEC_NS = None


def _build_nc():
    import concourse.bass as bass
    import concourse.bacc as bacc
    import concourse.mybir as mybir
    import concourse.tile as tile

    f32 = mybir.dt.float32
    bf16 = mybir.dt.bfloat16
    AF = mybir.ActivationFunctionType

    nc = bacc.Bacc(None, num_devices=NCORES)

    xT_d = nc.dram_tensor("xT", [DM, S], bf16, kind="ExternalInput")
    xqT_d = nc.dram_tensor("xqT", [DM, SH], bf16, kind="ExternalInput")
    cbw_d = nc.dram_tensor("cbw", [128, BFW_COLS], bf16, kind="ExternalInput")
    cbr_d = nc.dram_tensor("cbr", [128, BFR_COLS], bf16, kind="ExternalInput")
    cf_d = nc.dram_tensor("cf", [128, F_COLS], f32, kind="ExternalInput")
    out_d = nc.dram_tensor("out", [SH, DK], f32, kind="ExternalOutput")

    with tile.TileContext(nc) as tc, ExitStack() as ctx:
        singles = ctx.enter_context(tc.tile_pool(name="singles", bufs=1))
        psA = ctx.enter_context(tc.tile_pool(name="psA", bufs=1, space="PSUM"))
        psB = ctx.enter_context(tc.tile_pool(name="psB", bufs=2, space="PSUM"))
        epool = ctx.enter_context(tc.tile_pool(name="epool", bufs=3))

        # ---------------- packed constant loads ----------------
        cbw_sb = singles.tile([128, BFW_COLS], bf16)
        nc.sync.dma_start(out=cbw_sb, in_=cbw_d[:, :])
        cf_sb = singles.tile([128, F_COLS], f32)
        nc.sync.dma_start(out=cf_sb, in_=cf_d[:, :])
        cbr_sb = singles.tile([128, BFR_COLS], bf16)

        wkv_sb = cbw_sb[:, BF_WKV:BF_ID].rearrange("p (d c) -> p d c", d=ND)
        ident_sb = cbw_sb[:, BF_ID:BF_ID + 128]
        wq_sb = cbr_sb[:, BFR_WQ:BFR_MASK].rearrange("p (d c) -> p d c", d=ND)
        mask_sb = cbr_sb[:, BFR_MASK:BFR_COLS].rearrange(
            "p (kb q) -> p kb q", kb=NCORES)
        bkv_sb = cf_sb[:, 0:1]
        bq_sb = cf_sb[0:DK, 1:2]
        identf_sb = cf_sb[:, 2:2 + 128]

        xT_sb = singles.tile([128, ND, S], bf16)
        xq_sb = singles.tile([128, ND, SH], bf16)
        kT_sb = singles.tile([DK, S], bf16)
        vt_sb = singles.tile([128, NB, DK + 1], bf16)
        qT_sb = singles.tile([DK, SH], bf16)
        # ones column of V~ (denominator accumulator)
        nc.scalar.activation(vt_sb[:, :, DK:DK + 1], ident_sb[:, 0:NB],
                             AF.Identity, bias=1.0, scale=0.0)

        def load_cols(c0, c1):
            cs = slice(c0, c1)
            nc.sync.dma_start(
                out=xT_sb[:, :, cs],
                in_=xT_d[:, cs].rearrange("(d p) s -> p d s", p=128))

        def one_pass(rep):
            load_cols(0, 256)
            load_cols(256, 512)
            nc.sync.dma_start(out=xq_sb, in_=xqT_d[:, :].rearrange(
                "(d p) s -> p d s", p=128))
            nc.sync.dma_start(out=cbr_sb, in_=cbr_d[:, :])
            for g in range(1, NCH):
                load_cols(CHUNK * g, CHUNK * (g + 1))

            av_started = [False] * SLOTS
            av_ps = None

            for g in range(NCH):
                cs0 = slice(CHUNK * g, CHUNK * g + 256)
                cs1 = slice(CHUNK * g + 256, CHUNK * (g + 1))
                # two interleaved 256-col chains in ONE open accumulation
                # group (start on the very first matmul, stop on the last;
                # per-element has_written handles overwrite-vs-accumulate)
                kv_ps = psB.tile([128, 2, 256], f32, tag="kvps", bufs=1,
                                 name="kv_ps")
                for d in range(ND):
                    nc.tensor.matmul(kv_ps[:, 0, :], lhsT=wkv_sb[:, d, :],
                                     rhs=xT_sb[:, d, cs0],
                                     start=(d == 0), stop=False,
                                     skip_group_check=True)
                    nc.tensor.matmul(kv_ps[:, 1, :], lhsT=wkv_sb[:, d, :],
                                     rhs=xT_sb[:, d, cs1],
                                     start=False, stop=(d == ND - 1),
                                     skip_group_check=True)
                cs = slice(CHUNK * g, CHUNK * (g + 1))
                nc.vector.tensor_scalar_add(
                    kT_sb[:, cs].rearrange("k (h s) -> k h s", h=2),
                    kv_ps[0:DK, :, :], bkv_sb[0:DK, 0:1])
                vT_h = epool.tile([DK, CHUNK], f32, tag="vth", name="vT_h")
                nc.vector.tensor_scalar_add(
                    vT_h.rearrange("k (h s) -> k h s", h=2),
                    kv_ps[DK:128, :, :], bkv_sb[DK:128, 0:1])
                t_ps = psB.tile([128, 4, QB], f32, tag="scps", bufs=2,
                                name="t_ps")
                for sl in range(4):
                    nc.tensor.transpose(t_ps[:, sl, 0:DK],
                                        vT_h[:, QB * sl:QB * (sl + 1)],
                                        identf_sb[0:DK, 0:DK])
                nc.scalar.copy(vt_sb[:, 4 * g:4 * (g + 1), 0:DK],
                               t_ps[:, :, 0:DK])

                if g == 0:
                    # Q^T for own 512 rows (after chunk-0 proj in PE order)
                    q_ps = psA.tile([DK, SH], f32, name="q_ps", tag="qps")
                    for d in range(ND):
                        nc.tensor.matmul(q_ps, lhsT=wq_sb[:, d, :],
                                         rhs=xq_sb[:, d, :],
                                         start=(d == 0), stop=(d == ND - 1))
                    nc.scalar.activation(qT_sb, q_ps, AF.Identity,
                                         bias=bq_sb[:, 0:1], scale=1.0)
                    av_ps = [psA.tile([DK + 1, QB], f32, name=f"av{j}",
                                      tag=f"av{j}") for j in range(SLOTS)]

                # attention: every q-slot whose causal prefix includes chunk g
                slots = [j for j in range(SLOTS) if g <= 2 * j + 1]
                parts = {}
                for j in slots:
                    qc = slice(QB * j, QB * (j + 1))
                    sc_ps = psB.tile([128, 4, QB], f32, tag="scps",
                                     bufs=2, name="sc_ps")
                    e_sb = epool.tile([128, 4, QB], bf16, tag=f"e{j % 2}",
                                      name="e_sb")
                    for sl in range(4):
                        kb = 4 * g + sl
                        nc.tensor.matmul(sc_ps[:, sl, :],
                                         lhsT=kT_sb[:, QB * kb:QB * (kb + 1)],
                                         rhs=qT_sb[:, qc],
                                         start=True, stop=True)
                    nc.scalar.activation(e_sb, sc_ps, AF.Exp, scale=0.125)
                    if g >= 2 * j:  # diagonal band: mask (ones/tri/zeros by c)
                        mi = 4 * (g - 2 * j)
                        nc.vector.tensor_mul(e_sb, e_sb,
                                             mask_sb[:, mi:mi + 4, :])
                    parts[j] = (qc, e_sb)
                # AV round-robin across slots; each slot owns its own PSUM
                # bank (one open accumulation group per bank)
                for sl in range(4):
                    for j in slots:
                        qc, e_sb = parts[j]
                        kb = 4 * g + sl
                        last = (g == 2 * j + 1 and sl == 3)
                        nc.tensor.matmul(av_ps[j], lhsT=vt_sb[:, kb, :],
                                         rhs=e_sb[:, sl, :],
                                         start=(not av_started[j]), stop=last,
                                         skip_group_check=True)
                        av_started[j] = True

                # epilogue per finished slot: slot j's accumulation completes
                # at chunk 2j+1 -> normalize and store it while later chunks
                # stream (only slot 3 remains in the tail)
                if g % 2 == 1:
                    j = g // 2
                    qc = slice(QB * j, QB * (j + 1))
                    av_sb = epool.tile([DK + 1, QB], f32, tag="avsb",
                                       name="av_sb")
                    nc.scalar.copy(av_sb, av_ps[j])
                    t2 = psB.tile([128, 4, QB], f32, tag="scps", bufs=2,
                                  name="t2")
                    nc.tensor.transpose(t2[:, 0, 0:DK + 1], av_sb,
                                        identf_sb[0:DK + 1, 0:DK + 1])
                    rec = epool.tile([128, 1], f32, tag="rec", name="rec")
                    nc.vector.reciprocal(rec, t2[:, 0, DK:DK + 1])
                    out_sb = epool.tile([128, DK], f32, tag="osb",
                                        name="out_sb")
                    nc.vector.tensor_scalar_mul(out_sb, t2[:, 0, 0:DK], rec)
                    nc.sync.dma_start(out=out_d[QB * j:QB * (j + 1), :],
                                      in_=out_sb)

        for _rep in range(AMP):
            one_pass(_rep)

    nc.finalize()
    return nc


def _in_maps(x, Wq, bq, Wk, bk, Wv, bv):
    bf = ml_dtypes.bfloat16
    xT = np.ascontiguousarray(x.T).astype(bf)                      # [1024, 4096]
    tri = np.triu(np.ones((QB, QB), dtype=np.float32))  # E^T[k,q] valid iff k<=q

    # bf16 constant blobs
    wkvT = np.concatenate([Wk.T, Wv.T], axis=1)                    # [1024, 128]
    wkv_p = wkvT.reshape(ND, 128, 2 * DK).transpose(1, 0, 2).reshape(128, -1)
    wqT = Wq.T                                                     # [1024, 64]
    wq_p = wqT.reshape(ND, 128, DK).transpose(1, 0, 2).reshape(128, -1)
    ident = np.eye(128, dtype=np.float32)
    cbw = np.ascontiguousarray(
        np.concatenate([wkv_p, ident], axis=1).astype(bf))
    assert cbw.shape == (128, BFW_COLS)

    # f32 constant blob [128, F_COLS]
    cf = np.zeros((128, F_COLS), dtype=np.float32)
    cf[:, 0] = np.concatenate([bk, bv])
    cf[0:DK, 1] = bq
    cf[:, 2:2 + 128] = ident

    maps = []
    for c in range(NCORES):
        rows = np.concatenate([np.arange(QB * (8 * sl + c),
                                         QB * (8 * sl + c) + QB)
                               for sl in range(SLOTS)])
        xqT = np.ascontiguousarray(x[rows].T).astype(bf)           # [1024, 512]
        # diagonal-band mask: key index k within band vs own position c
        m = np.zeros((NCORES, QB, QB), dtype=np.float32)
        m[:c] = 1.0
        m[c] = tri
        mask_p = m.transpose(1, 0, 2).reshape(128, -1)
        cbr = np.ascontiguousarray(
            np.concatenate([wq_p, mask_p], axis=1).astype(bf))
        assert cbr.shape == (128, BFR_COLS)
        maps.append({"xT": xT, "xqT": xqT, "cbw": cbw, "cbr": cbr,
                     "cf": cf})
    return maps


def kernel(**inputs):
    global LAST_EXEC_NS
    x = np.asarray(inputs["x"], dtype=np.float32)
    args = [np.asarray(inputs[k], dtype=np.float32)
            for k in ("Wq", "bq", "Wk", "bk", "Wv", "bv")]
    in_maps = _in_maps(x, args[0], args[1], args[2], args[3], args[4], args[5])

    nc = _build_nc()
    from concourse.bass_utils import run_bass_kernel_spmd
    res = run_bass_kernel_spmd(nc, in_maps, core_ids=list(range(NCORES)))
    LAST_EXEC_NS = res.exec_time_ns

    out = np.zeros((S, DK), dtype=np.float32)
    for c in range(NCORES):
        r = res.results[c]["out"]
        for sl in range(SLOTS):
            b = 8 * sl + c
            out[QB * b:QB * (b + 1)] = r[QB * sl:QB * (sl + 1)]
    return out


# revision 14
# speedup vs baseline: 2.6339x; 1.2340x over previous
"""Causal single-head attention on 8 Trainium2 NeuronCores — fully local.

Problem: x[4096,1024] -> Q,K,V = x@W.T+b (d_k=64), out = softmax(causal(QK^T/8)) @ V.

Strategy (replicated K/V, zero communication):
  - Every core loads the FULL x^T in bf16 (8 MB) and computes K^T and V for
    all 4096 rows locally; no collective, no cross-core sync of any kind.
    The 8 MB stream overlaps the projection/attention pipeline.
  - Query blocks of 128 rows; core c owns global blocks {c, 8+c, 16+c, 24+c}
    (strided) -> every core runs the IDENTICAL program. Slot j attends key
    blocks 0..8j+7 (uniform); within the diagonal band (blocks 8j..8j+7) a
    per-core host-built mask (ones/tri/zeros by key index vs c) enforces
    exact causality. Off-band blocks are always fully valid -> no masking.
  - x-column chunks of 512 stream in; chunk g yields key blocks 4g..4g+3.
    K^T/V^T come from a stacked [Wk|Wv] projection run as TWO interleaved
    256-column PSUM chains (keeps the PE p-state ramp hot); V~ blocks are PE
    transposes of V^T with a ones-column appended so the AV matmul also
    accumulates the softmax denominator.
  - Attention (scores -> exp -> mask -> AV) for every (q-slot, key-group)
    pair runs as soon as its chunk is projected; AV matmuls are emitted
    round-robin across slots so accumulation chains interleave on the PE.
  - All attention matmuls in bf16 (rate-1 at any moving width); exp on
    ScalarE with the 1/8 scale folded in; accumulation in f32 PSUM.
  - Constants ride in two packed blobs (one bf16, one f32) to amortize
    per-DMA overhead; the x stream is ordered chunk0 -> xq -> chunks 1-7 so
    the projection pipeline starts as early as possible.
"""

import os
import numpy as np
import ml_dtypes
from contextlib import ExitStack

S, DM, DK = 4096, 1024, 64
NCORES = 8
QB = 128                      # rows per block
SLOTS = 4                     # q-blocks per core
SH = QB * SLOTS               # 512 own query rows per core
NB = S // QB                  # 32 key blocks
CHUNK = 512                   # x columns per streamed chunk
NCH = S // CHUNK              # 8 chunks
ND = DM // 128                # 8 contraction chunks

# cb_w bf16 blob (early): wkv [8, 128] | ident [128]
BF_WKV = 0
BF_ID = ND * 128
BFW_COLS = BF_ID + 128
# cb_r bf16 blob (later): wq [8, 64] | mask [8, 128]
BFR_WQ = 0
BFR_MASK = ND * DK
BFR_COLS = BFR_MASK + NCORES * QB
# f32 blob layout: bkv [1] | bq [1] | identf [128]
F_COLS = 2 + 128

AMP = int(os.environ.get("KERNEL_AMP", "1"))  # repeat whole pipeline in-NEFF

LAST_EXEC_NS = None


def _build_nc():
    import concourse.bass as bass
    import concourse.bacc as bacc
    import concourse.mybir as mybir
    import concourse.tile as tile

    f32 = mybir.dt.float32
    bf16 = mybir.dt.bfloat16
    AF = mybir.ActivationFunctionType

    nc = bacc.Bacc(None, num_devices=NCORES)

    xT_d = nc.dram_tensor("xT", [DM, S], bf16, kind="ExternalInput")
    xqT_d = nc.dram_tensor("xqT", [DM, SH], bf16, kind="ExternalInput")
    cbw_d = nc.dram_tensor("cbw", [128, BFW_COLS], bf16, kind="ExternalInput")
    cbr_d = nc.dram_tensor("cbr", [128, BFR_COLS], bf16, kind="ExternalInput")
    cf_d = nc.dram_tensor("cf", [128, F_COLS], f32, kind="ExternalInput")
    out_d = nc.dram_tensor("out", [SH, DK], f32, kind="ExternalOutput")

    with tile.TileContext(nc) as tc, ExitStack() as ctx:
        singles = ctx.enter_context(tc.tile_pool(name="singles", bufs=1))
        psA = ctx.enter_context(tc.tile_pool(name="psA", bufs=1, space="PSUM"))
        psB = ctx.enter_context(tc.tile_pool(name="psB", bufs=2, space="PSUM"))
        epool = ctx.enter_context(tc.tile_pool(name="epool", bufs=3))

        # ---------------- packed constant loads ----------------
        cbw_sb = singles.tile([128, BFW_COLS], bf16)
        nc.sync.dma_start(out=cbw_sb, in_=cbw_d[:, :])
        cf_sb = singles.tile([128, F_COLS], f32)
        nc.sync.dma_start(out=cf_sb, in_=cf_d[:, :])
        cbr_sb = singles.tile([128, BFR_COLS], bf16)

        wkv_sb = cbw_sb[:, BF_WKV:BF_ID].rearrange("p (d c) -> p d c", d=ND)
        ident_sb = cbw_sb[:, BF_ID:BF_ID + 128]
        wq_sb = cbr_sb[:, BFR_WQ:BFR_MASK].rearrange("p (d c) -> p d c", d=ND)
        mask_sb = cbr_sb[:, BFR_MASK:BFR_COLS].rearrange(
            "p (kb q) -> p kb q", kb=NCORES)
        bkv_sb = cf_sb[:, 0:1]
        bq_sb = cf_sb[0:DK, 1:2]
        identf_sb = cf_sb[:, 2:2 + 128]

        xT_sb = singles.tile([128, ND, S], bf16)
        xq_sb = singles.tile([128, ND, SH], bf16)
        kT_sb = singles.tile([DK, S], bf16)
        vt_sb = singles.tile([128, NB, DK + 1], bf16)
        qT_sb = singles.tile([DK, SH], bf16)
        # ones column of V~ (denominator accumulator)
        nc.scalar.activation(vt_sb[:, :, DK:DK + 1], ident_sb[:, 0:NB],
                             AF.Identity, bias=1.0, scale=0.0)

        def load_cols(c0, c1):
            cs = slice(c0, c1)
            nc.sync.dma_start(
                out=xT_sb[:, :, cs],
                in_=xT_d[:, cs].rearrange("(d p) s -> p d s", p=128))

        def one_pass(rep):
            load_cols(0, 256)
            load_cols(256, 512)
            nc.sync.dma_start(out=xq_sb, in_=xqT_d[:, :].rearrange(
                "(d p) s -> p d s", p=128))
            nc.sync.dma_start(out=cbr_sb, in_=cbr_d[:, :])
            for g in range(1, NCH):
                load_cols(CHUNK * g, CHUNK * (g + 1))

            av_started = [False] * SLOTS
            av_ps = None

            for g in range(NCH):
                cs0 = slice(CHUNK * g, CHUNK * g + 256)
                cs1 = slice(CHUNK * g + 256, CHUNK * (g + 1))
                # two interleaved 256-col chains in ONE open accumulation
                # group (start on the very first matmul, stop on the last;
                # per-element has_written handles overwrite-vs-accumulate)
                kv_ps = psB.tile([128, 2, 256], f32, tag="kvps", bufs=2,
                                 name="kv_ps")
                for d in range(ND):
                    nc.tensor.matmul(kv_ps[:, 0, :], lhsT=wkv_sb[:, d, :],
                                     rhs=xT_sb[:, d, cs0],
                                     start=(d == 0), stop=False,
                                     skip_group_check=True)
                    nc.tensor.matmul(kv_ps[:, 1, :], lhsT=wkv_sb[:, d, :],
                                     rhs=xT_sb[:, d, cs1],
                                     start=False, stop=(d == ND - 1),
                                     skip_group_check=True)
                cs = slice(CHUNK * g, CHUNK * (g + 1))
                nc.vector.tensor_scalar_add(
                    kT_sb[:, cs].rearrange("k (h s) -> k h s", h=2),
                    kv_ps[0:DK, :, :], bkv_sb[0:DK, 0:1])
                vT_h = epool.tile([DK, CHUNK], f32, tag="vth", name="vT_h")
                nc.vector.tensor_scalar_add(
                    vT_h.rearrange("k (h s) -> k h s", h=2),
                    kv_ps[DK:128, :, :], bkv_sb[DK:128, 0:1])
                t_ps = psB.tile([128, 4, QB], f32, tag="scps", bufs=2,
                                name="t_ps")
                for sl in range(4):
                    nc.tensor.transpose(t_ps[:, sl, 0:DK],
                                        vT_h[:, QB * sl:QB * (sl + 1)],
                                        identf_sb[0:DK, 0:DK])
                nc.scalar.copy(vt_sb[:, 4 * g:4 * (g + 1), 0:DK],
                               t_ps[:, :, 0:DK])

                if g == 0:
                    # Q^T for own 512 rows (after chunk-0 proj in PE order)
                    q_ps = psA.tile([DK, SH], f32, name="q_ps", tag="qps")
                    for d in range(ND):
                        nc.tensor.matmul(q_ps, lhsT=wq_sb[:, d, :],
                                         rhs=xq_sb[:, d, :],
                                         start=(d == 0), stop=(d == ND - 1))
                    nc.scalar.activation(qT_sb, q_ps, AF.Identity,
                                         bias=bq_sb[:, 0:1], scale=1.0)
                    av_acc = singles.tile([DK + 1, SLOTS, QB], f32,
                                          name="av_acc")

                # attention: every q-slot whose causal prefix includes chunk g
                slots = [j for j in range(SLOTS) if g <= 2 * j + 1]
                parts = {}
                for j in slots:
                    qc = slice(QB * j, QB * (j + 1))
                    sc_ps = psB.tile([128, 4, QB], f32, tag="scps",
                                     bufs=2, name="sc_ps")
                    e_sb = epool.tile([128, 4, QB], bf16, tag=f"e{j % 2}",
                                      name="e_sb")
                    for sl in range(4):
                        kb = 4 * g + sl
                        nc.tensor.matmul(sc_ps[:, sl, :],
                                         lhsT=kT_sb[:, QB * kb:QB * (kb + 1)],
                                         rhs=qT_sb[:, qc],
                                         start=True, stop=True)
                    nc.scalar.activation(e_sb, sc_ps, AF.Exp, scale=0.125)
                    if g >= 2 * j:  # diagonal band: mask (ones/tri/zeros by c)
                        mi = 4 * (g - 2 * j)
                        nc.vector.tensor_mul(e_sb, e_sb,
                                             mask_sb[:, mi:mi + 4, :])
                    parts[j] = (qc, e_sb)
                # AV as one CLOSED accumulation group per (chunk, slot) in a
                # rotating scratch bank, then accumulated into SBUF on DVE:
                # one open group per bank at all times.
                for j in slots:
                    qc, e_sb = parts[j]
                    avp = psB.tile([DK + 1, QB], f32, tag="avp", bufs=2,
                                   name="avp")
                    for sl in range(4):
                        kb = 4 * g + sl
                        nc.tensor.matmul(avp, lhsT=vt_sb[:, kb, :],
                                         rhs=e_sb[:, sl, :],
                                         start=(sl == 0), stop=(sl == 3),
                                         skip_group_check=True)
                    if not av_started[j]:
                        nc.vector.tensor_copy(av_acc[:, j, :], avp)
                        av_started[j] = True
                    else:
                        nc.vector.tensor_tensor(av_acc[:, j, :],
                                                av_acc[:, j, :], avp,
                                                op=mybir.AluOpType.add)

                # epilogue per finished slot: slot j's accumulation completes
                # at chunk 2j+1 -> normalize and store it while later chunks
                # stream (only slot 3 remains in the tail)
                if g % 2 == 1:
                    j = g // 2
                    qc = slice(QB * j, QB * (j + 1))
                    t2 = psB.tile([128, 4, QB], f32, tag="scps", bufs=2,
                                  name="t2")
                    nc.tensor.transpose(t2[:, 0, 0:DK + 1], av_acc[:, j, :],
                                        identf_sb[0:DK + 1, 0:DK + 1])
                    rec = epool.tile([128, 1], f32, tag="rec", name="rec")
                    nc.vector.reciprocal(rec, t2[:, 0, DK:DK + 1])
                    out_sb = epool.tile([128, DK], f32, tag="osb",
                                        name="out_sb")
                    nc.vector.tensor_scalar_mul(out_sb, t2[:, 0, 0:DK], rec)
                    nc.sync.dma_start(out=out_d[QB * j:QB * (j + 1), :],
                                      in_=out_sb)

        for _rep in range(AMP):
            one_pass(_rep)

    nc.finalize()
    return nc


def _in_maps(x, Wq, bq, Wk, bk, Wv, bv):
    bf = ml_dtypes.bfloat16
    xT = np.ascontiguousarray(x.T).astype(bf)                      # [1024, 4096]
    tri = np.triu(np.ones((QB, QB), dtype=np.float32))  # E^T[k,q] valid iff k<=q

    # bf16 constant blobs
    wkvT = np.concatenate([Wk.T, Wv.T], axis=1)                    # [1024, 128]
    wkv_p = wkvT.reshape(ND, 128, 2 * DK).transpose(1, 0, 2).reshape(128, -1)
    wqT = Wq.T                                                     # [1024, 64]
    wq_p = wqT.reshape(ND, 128, DK).transpose(1, 0, 2).reshape(128, -1)
    ident = np.eye(128, dtype=np.float32)
    cbw = np.ascontiguousarray(
        np.concatenate([wkv_p, ident], axis=1).astype(bf))
    assert cbw.shape == (128, BFW_COLS)

    # f32 constant blob [128, F_COLS]
    cf = np.zeros((128, F_COLS), dtype=np.float32)
    cf[:, 0] = np.concatenate([bk, bv])
    cf[0:DK, 1] = bq
    cf[:, 2:2 + 128] = ident

    maps = []
    for c in range(NCORES):
        rows = np.concatenate([np.arange(QB * (8 * sl + c),
                                         QB * (8 * sl + c) + QB)
                               for sl in range(SLOTS)])
        xqT = np.ascontiguousarray(x[rows].T).astype(bf)           # [1024, 512]
        # diagonal-band mask: key index k within band vs own position c
        m = np.zeros((NCORES, QB, QB), dtype=np.float32)
        m[:c] = 1.0
        m[c] = tri
        mask_p = m.transpose(1, 0, 2).reshape(128, -1)
        cbr = np.ascontiguousarray(
            np.concatenate([wq_p, mask_p], axis=1).astype(bf))
        assert cbr.shape == (128, BFR_COLS)
        maps.append({"xT": xT, "xqT": xqT, "cbw": cbw, "cbr": cbr,
                     "cf": cf})
    return maps


def kernel(**inputs):
    global LAST_EXEC_NS
    x = np.asarray(inputs["x"], dtype=np.float32)
    args = [np.asarray(inputs[k], dtype=np.float32)
            for k in ("Wq", "bq", "Wk", "bk", "Wv", "bv")]
    in_maps = _in_maps(x, args[0], args[1], args[2], args[3], args[4], args[5])

    nc = _build_nc()
    from concourse.bass_utils import run_bass_kernel_spmd
    res = run_bass_kernel_spmd(nc, in_maps, core_ids=list(range(NCORES)))
    LAST_EXEC_NS = res.exec_time_ns

    out = np.zeros((S, DK), dtype=np.float32)
    for c in range(NCORES):
        r = res.results[c]["out"]
        for sl in range(SLOTS):
            b = 8 * sl + c
            out[QB * b:QB * (b + 1)] = r[QB * sl:QB * (sl + 1)]
    return out


# revision 21
# speedup vs baseline: 2.6360x; 1.0008x over previous
"""Causal single-head attention on 8 Trainium2 NeuronCores — fully local.

Problem: x[4096,1024] -> Q,K,V = x@W.T+b (d_k=64), out = softmax(causal(QK^T/8)) @ V.

Strategy (replicated K/V, zero communication):
  - Every core loads the FULL x^T in bf16 (8 MB) and computes K^T and V for
    all 4096 rows locally; no collective, no cross-core sync of any kind.
    The 8 MB stream overlaps the projection/attention pipeline.
  - Query blocks of 128 rows; core c owns global blocks {c, 8+c, 16+c, 24+c}
    (strided) -> every core runs the IDENTICAL program. Slot j attends key
    blocks 0..8j+7 (uniform); within the diagonal band (blocks 8j..8j+7) a
    per-core host-built mask (ones/tri/zeros by key index vs c) enforces
    exact causality. Off-band blocks are always fully valid -> no masking.
  - x-column chunks of 512 stream in; chunk g yields key blocks 4g..4g+3.
    K^T/V^T come from a stacked [Wk|Wv] projection run as TWO interleaved
    256-column PSUM chains (keeps the PE p-state ramp hot); V~ blocks are PE
    transposes of V^T with a ones-column appended so the AV matmul also
    accumulates the softmax denominator.
  - Attention (scores -> exp -> mask -> AV) for every (q-slot, key-group)
    pair runs as soon as its chunk is projected; AV matmuls are emitted
    round-robin across slots so accumulation chains interleave on the PE.
  - All attention matmuls in bf16 (rate-1 at any moving width); exp on
    ScalarE with the 1/8 scale folded in; accumulation in f32 PSUM.
  - Constants ride in two packed blobs (one bf16, one f32) to amortize
    per-DMA overhead; the x stream is ordered chunk0 -> xq -> chunks 1-7 so
    the projection pipeline starts as early as possible.
"""

import os
import numpy as np
import ml_dtypes
from contextlib import ExitStack

S, DM, DK = 4096, 1024, 64
NCORES = 8
QB = 128                      # rows per block
SLOTS = 4                     # q-blocks per core
SH = QB * SLOTS               # 512 own query rows per core
NB = S // QB                  # 32 key blocks
CHUNK = 512                   # x columns per streamed chunk
NCH = S // CHUNK              # 8 chunks
ND = DM // 128                # 8 contraction chunks

# cb_w bf16 blob (early): wkv [8, 128] | ident [128]
BF_WKV = 0
BF_ID = ND * 128
BFW_COLS = BF_ID + 128
# cb_r bf16 blob (later): wq [8, 64] | mask [8, 128]
BFR_WQ = 0
BFR_MASK = ND * DK
BFR_COLS = BFR_MASK + NCORES * QB
# f32 blob layout: bkv [1] | bq [1] | identf [128]
F_COLS = 2 + 128

AMP = int(os.environ.get("KERNEL_AMP", "1"))  # repeat whole pipeline in-NEFF

LAST_EXEC_NS = None


def _build_nc():
    import concourse.bass as bass
    import concourse.bacc as bacc
    import concourse.mybir as mybir
    import concourse.tile as tile

    f32 = mybir.dt.float32
    bf16 = mybir.dt.bfloat16
    AF = mybir.ActivationFunctionType

    nc = bacc.Bacc(None, num_devices=NCORES)

    xT_d = nc.dram_tensor("xT", [DM, S], bf16, kind="ExternalInput")
    xqT_d = nc.dram_tensor("xqT", [DM, SH], bf16, kind="ExternalInput")
    cbw_d = nc.dram_tensor("cbw", [128, BFW_COLS], bf16, kind="ExternalInput")
    cbr_d = nc.dram_tensor("cbr", [128, BFR_COLS], bf16, kind="ExternalInput")
    cf_d = nc.dram_tensor("cf", [128, F_COLS], f32, kind="ExternalInput")
    out_d = nc.dram_tensor("out", [SH, DK], f32, kind="ExternalOutput")

    with tile.TileContext(nc) as tc, ExitStack() as ctx:
        singles = ctx.enter_context(tc.tile_pool(name="singles", bufs=1))
        psA = ctx.enter_context(tc.tile_pool(name="psA", bufs=1, space="PSUM"))
        psB = ctx.enter_context(tc.tile_pool(name="psB", bufs=2, space="PSUM"))
        epool = ctx.enter_context(tc.tile_pool(name="epool", bufs=4))

        # ---------------- packed constant loads ----------------
        cbw_sb = singles.tile([128, BFW_COLS], bf16)
        nc.sync.dma_start(out=cbw_sb, in_=cbw_d[:, :])
        cf_sb = singles.tile([128, F_COLS], f32)
        nc.sync.dma_start(out=cf_sb, in_=cf_d[:, :])
        cbr_sb = singles.tile([128, BFR_COLS], bf16)

        wkv_sb = cbw_sb[:, BF_WKV:BF_ID].rearrange("p (d c) -> p d c", d=ND)
        ident_sb = cbw_sb[:, BF_ID:BF_ID + 128]
        wq_sb = cbr_sb[:, BFR_WQ:BFR_MASK].rearrange("p (d c) -> p d c", d=ND)
        mask_sb = cbr_sb[:, BFR_MASK:BFR_COLS].rearrange(
            "p (kb q) -> p kb q", kb=NCORES)
        bkv_sb = cf_sb[:, 0:1]
        bq_sb = cf_sb[0:DK, 1:2]
        identf_sb = cf_sb[:, 2:2 + 128]

        xT_sb = singles.tile([128, ND, S], bf16)
        xq_sb = singles.tile([128, ND, SH], bf16)
        kT_sb = singles.tile([DK, S], bf16)
        vt_sb = singles.tile([128, NB, DK + 1], bf16)
        qT_sb = singles.tile([DK, SH], bf16)
        # ones column of V~ (denominator accumulator)
        nc.scalar.activation(vt_sb[:, :, DK:DK + 1], ident_sb[:, 0:NB],
                             AF.Identity, bias=1.0, scale=0.0)

        def load_cols(c0, c1):
            cs = slice(c0, c1)
            nc.sync.dma_start(
                out=xT_sb[:, :, cs],
                in_=xT_d[:, cs].rearrange("(d p) s -> p d s", p=128))

        def one_pass(rep):
            load_cols(0, 256)
            load_cols(256, 512)
            nc.sync.dma_start(out=xq_sb, in_=xqT_d[:, :].rearrange(
                "(d p) s -> p d s", p=128))
            nc.sync.dma_start(out=cbr_sb, in_=cbr_d[:, :])
            for g in range(1, NCH):
                load_cols(CHUNK * g, CHUNK * (g + 1))

            av_started = [False] * SLOTS
            av_ps = None

            for g in range(NCH):
                cs0 = slice(CHUNK * g, CHUNK * g + 256)
                cs1 = slice(CHUNK * g + 256, CHUNK * (g + 1))
                # two interleaved 256-col chains in ONE open accumulation
                # group (start on the very first matmul, stop on the last;
                # per-element has_written handles overwrite-vs-accumulate)
                kv_ps = psB.tile([128, 2, 256], f32, tag="kvps", bufs=2,
                                 name="kv_ps")
                for d in range(ND):
                    nc.tensor.matmul(kv_ps[:, 0, :], lhsT=wkv_sb[:, d, :],
                                     rhs=xT_sb[:, d, cs0],
                                     start=(d == 0), stop=False,
                                     skip_group_check=True)
                    nc.tensor.matmul(kv_ps[:, 1, :], lhsT=wkv_sb[:, d, :],
                                     rhs=xT_sb[:, d, cs1],
                                     start=False, stop=(d == ND - 1),
                                     skip_group_check=True)
                cs = slice(CHUNK * g, CHUNK * (g + 1))
                nc.vector.tensor_scalar_add(
                    kT_sb[:, cs].rearrange("k (h s) -> k h s", h=2),
                    kv_ps[0:DK, :, :], bkv_sb[0:DK, 0:1])
                vT_h = epool.tile([DK, CHUNK], f32, tag="vth", name="vT_h")
                nc.vector.tensor_scalar_add(
                    vT_h.rearrange("k (h s) -> k h s", h=2),
                    kv_ps[DK:128, :, :], bkv_sb[DK:128, 0:1])
                t_ps = psB.tile([128, 4, QB], f32, tag="scps", bufs=3,
                                name="t_ps")
                for sl in range(4):
                    nc.tensor.transpose(t_ps[:, sl, 0:DK],
                                        vT_h[:, QB * sl:QB * (sl + 1)],
                                        identf_sb[0:DK, 0:DK])
                nc.scalar.copy(vt_sb[:, 4 * g:4 * (g + 1), 0:DK],
                               t_ps[:, :, 0:DK])

                if g == 0:
                    # Q^T for own 512 rows (after chunk-0 proj in PE order)
                    q_ps = psA.tile([DK, SH], f32, name="q_ps", tag="qps")
                    for d in range(ND):
                        nc.tensor.matmul(q_ps, lhsT=wq_sb[:, d, :],
                                         rhs=xq_sb[:, d, :],
                                         start=(d == 0), stop=(d == ND - 1))
                    nc.scalar.activation(qT_sb, q_ps, AF.Identity,
                                         bias=bq_sb[:, 0:1], scale=1.0)
                    av_acc = [singles.tile([DK + 1, QB], f32,
                                            name=f"av_acc{j}")
                              for j in range(SLOTS)]

                # attention: every q-slot whose causal prefix includes chunk g
                slots = [j for j in range(SLOTS) if g <= 2 * j + 1]
                parts = {}
                for j in slots:
                    qc = slice(QB * j, QB * (j + 1))
                    sc_ps = psB.tile([128, 4, QB], f32, tag="scps",
                                     bufs=3, name="sc_ps")
                    e_sb = epool.tile([128, 4, QB], bf16, tag=f"e{j % 2}",
                                      name="e_sb")
                    for sl in range(4):
                        kb = 4 * g + sl
                        nc.tensor.matmul(sc_ps[:, sl, :],
                                         lhsT=kT_sb[:, QB * kb:QB * (kb + 1)],
                                         rhs=qT_sb[:, qc],
                                         start=True, stop=True)
                    nc.scalar.activation(e_sb, sc_ps, AF.Exp, scale=0.125)
                    if g >= 2 * j:  # diagonal band: mask (ones/tri/zeros by c)
                        mi = 4 * (g - 2 * j)
                        nc.vector.tensor_mul(e_sb, e_sb,
                                             mask_sb[:, mi:mi + 4, :])
                    parts[j] = (qc, e_sb)
                # AV as one CLOSED accumulation group per (chunk, slot) in a
                # rotating scratch bank, then accumulated into SBUF on DVE:
                # one open group per bank at all times.
                for j in slots:
                    qc, e_sb = parts[j]
                    avp = psB.tile([DK + 1, QB], f32, tag="avp", bufs=2,
                                   name="avp")
                    for sl in range(4):
                        kb = 4 * g + sl
                        nc.tensor.matmul(avp, lhsT=vt_sb[:, kb, :],
                                         rhs=e_sb[:, sl, :],
                                         start=(sl == 0), stop=(sl == 3),
                                         skip_group_check=True)
                    if not av_started[j]:
                        nc.vector.tensor_copy(av_acc[j], avp)
                        av_started[j] = True
                    else:
                        nc.vector.tensor_tensor(av_acc[j], av_acc[j], avp,
                                                op=mybir.AluOpType.add)

                # epilogue per finished slot: slot j's accumulation completes
                # at chunk 2j+1 -> normalize and store it while later chunks
                # stream (only slot 3 remains in the tail)
                if g % 2 == 1:
                    j = g // 2
                    qc = slice(QB * j, QB * (j + 1))
                    t2 = psB.tile([128, 4, QB], f32, tag="scps", bufs=3,
                                  name="t2")
                    nc.tensor.transpose(t2[:, 0, 0:DK + 1], av_acc[j],
                                        identf_sb[0:DK + 1, 0:DK + 1])
                    rec = epool.tile([128, 1], f32, tag="rec", name="rec")
                    nc.vector.reciprocal(rec, t2[:, 0, DK:DK + 1])
                    out_sb = epool.tile([128, DK], f32, tag="osb",
                                        name="out_sb")
                    nc.vector.tensor_scalar_mul(out_sb, t2[:, 0, 0:DK], rec)
                    nc.sync.dma_start(out=out_d[QB * j:QB * (j + 1), :],
                                      in_=out_sb)

        for _rep in range(AMP):
            one_pass(_rep)

    nc.finalize()
    return nc


def _in_maps(x, Wq, bq, Wk, bk, Wv, bv):
    bf = ml_dtypes.bfloat16
    xT = np.ascontiguousarray(x.T).astype(bf)                      # [1024, 4096]
    tri = np.triu(np.ones((QB, QB), dtype=np.float32))  # E^T[k,q] valid iff k<=q

    # bf16 constant blobs
    wkvT = np.concatenate([Wk.T, Wv.T], axis=1)                    # [1024, 128]
    wkv_p = wkvT.reshape(ND, 128, 2 * DK).transpose(1, 0, 2).reshape(128, -1)
    wqT = Wq.T                                                     # [1024, 64]
    wq_p = wqT.reshape(ND, 128, DK).transpose(1, 0, 2).reshape(128, -1)
    ident = np.eye(128, dtype=np.float32)
    cbw = np.ascontiguousarray(
        np.concatenate([wkv_p, ident], axis=1).astype(bf))
    assert cbw.shape == (128, BFW_COLS)

    # f32 constant blob [128, F_COLS]
    cf = np.zeros((128, F_COLS), dtype=np.float32)
    cf[:, 0] = np.concatenate([bk, bv])
    cf[0:DK, 1] = bq
    cf[:, 2:2 + 128] = ident

    maps = []
    for c in range(NCORES):
        rows = np.concatenate([np.arange(QB * (8 * sl + c),
                                         QB * (8 * sl + c) + QB)
                               for sl in range(SLOTS)])
        xqT = np.ascontiguousarray(x[rows].T).astype(bf)           # [1024, 512]
        # diagonal-band mask: key index k within band vs own position c
        m = np.zeros((NCORES, QB, QB), dtype=np.float32)
        m[:c] = 1.0
        m[c] = tri
        mask_p = m.transpose(1, 0, 2).reshape(128, -1)
        cbr = np.ascontiguousarray(
            np.concatenate([wq_p, mask_p], axis=1).astype(bf))
        assert cbr.shape == (128, BFR_COLS)
        maps.append({"xT": xT, "xqT": xqT, "cbw": cbw, "cbr": cbr,
                     "cf": cf})
    return maps


def kernel(**inputs):
    global LAST_EXEC_NS
    x = np.asarray(inputs["x"], dtype=np.float32)
    args = [np.asarray(inputs[k], dtype=np.float32)
            for k in ("Wq", "bq", "Wk", "bk", "Wv", "bv")]
    in_maps = _in_maps(x, args[0], args[1], args[2], args[3], args[4], args[5])

    nc = _build_nc()
    from concourse.bass_utils import run_bass_kernel_spmd
    res = run_bass_kernel_spmd(nc, in_maps, core_ids=list(range(NCORES)))
    LAST_EXEC_NS = res.exec_time_ns

    out = np.zeros((S, DK), dtype=np.float32)
    for c in range(NCORES):
        r = res.results[c]["out"]
        for sl in range(SLOTS):
            b = 8 * sl + c
            out[QB * b:QB * (b + 1)] = r[QB * sl:QB * (sl + 1)]
    return out
